# revision 7
# baseline (speedup 1.0000x reference)
"""Trainium2 kernel for nn_AdaOpenController.

Reference semantics (G=4096 groups, P=4 pairs, 2 muscles, L=1024 dofs):
    w   = tanh(weight[step])                  # (G,)
    mu  = [relu(-w), relu(w)]                 # (G, 2) per-group segment heads
    out = 1 - concat([mu, (1-prev_a)[..., :-1]], -1)
i.e. for each of the G*P*2 length-L segments:
    out[seg, 0]  = 1 - mu[g, c]      (c = muscle index, segment head)
    out[seg, l]  = prev_a[seg, l-1]  (l >= 1; pure shift-by-one copy)

Memory-bound: 128 MiB in + 128 MiB out. Sharded data-parallel over the
group axis G across 8 NeuronCores (16 MiB in/out per core).

Raw Bass (no Tile): walrus rejects queue DMAs with >1 embedded sync
wait, so all cross-engine ordering uses standalone wait_ge sequencer
instructions and explicit semaphores. Per core, 4 tiles of
[128 part x 8 seg x 1024]:
  SP    : wcol load, then the 4 strided tile loads (data lands at
          free-offset +1 = the shift) -> in_sem[t]
  ACT   : tanh(wcol)                                  -> act_sem
  DVE   : head values a0/a1 -> vals; per tile wait in_sem[t], write the
          8 segment heads                             -> dve_sem
  POOL  : per tile wait dve_sem>=t+1 (transitively covers in_sem[t]
          via DVE program order), contiguous store; final drain.
"""

import sys

if "/opt/trn_rl_repo" not in sys.path:
    sys.path.insert(0, "/opt/trn_rl_repo")

from contextlib import ExitStack

import numpy as np

CTRL_NUM = 256
G = 4096                 # muscle groups
P = 4                    # shared muscle pairs per group
L = 1024                 # segments (act dofs) per muscle
M = 8                    # NeuronCores
G_LOC = G // M           # 512 groups per core
SEGS = P * 2             # 8 length-L segments per group
N_TILES = G_LOC // 128   # 4 tiles per core, one group per partition
LOC = G_LOC * SEGS * L   # elements per core

_NC_CACHE = None
TRACE = False            # test harness sets True for neuron-profile
LAST_RESULT = None       # stashes the BassKernelResults when TRACE


def _build():
    import concourse.bass as bass
    import concourse.mybir as mybir

    dt = mybir.dt.float32
    nc = bass.Bass()
    prev = nc.declare_dram_parameter(
        "prev", [N_TILES, 128, SEGS, L], dt, isOutput=False
    )
    wcol = nc.declare_dram_parameter("wcol", [128, N_TILES], dt, isOutput=False)
    out = nc.declare_dram_parameter("out", [N_TILES, 128, SEGS, L], dt, isOutput=True)

    with ExitStack() as ctx:
        ec = ctx.enter_context
        wc = ec(nc.sbuf_tensor("wc", [128, N_TILES], dt))
        wt = ec(nc.sbuf_tensor("wt", [128, N_TILES], dt))
        a0 = ec(nc.sbuf_tensor("a0", [128, N_TILES], dt))
        nw = ec(nc.sbuf_tensor("nw", [128, N_TILES], dt))
        a1 = ec(nc.sbuf_tensor("a1", [128, N_TILES], dt))
        vals = ec(nc.sbuf_tensor("vals", [128, N_TILES, SEGS], dt))
        tiles = [
            ec(nc.sbuf_tensor(f"tile{t}", [128, SEGS, L], dt))
            for t in range(N_TILES)
        ]
        w_sem = ec(nc.semaphore("w_sem"))
        act_sem = ec(nc.semaphore("act_sem"))
        in_sems = [ec(nc.semaphore(f"in_sem{t}")) for t in range(N_TILES)]
        p_sem = ec(nc.semaphore("p_sem"))
        dve_sem = ec(nc.semaphore("dve_sem"))
        out_sem = ec(nc.semaphore("out_sem"))

        with nc.Block() as block:

            @block.sync
            def _(sync):
                sync.dma_start(out=wc[:], in_=wcol[:, :]).then_inc(w_sem, 16)
                for t in range(N_TILES):
                    # shifted load: segment element l-1 lands at free slot l
                    sync.dma_start(
                        out=tiles[t][:, :, 1:L], in_=prev[t, :, :, 0 : L - 1]
                    ).then_inc(in_sems[t], 16)

            @block.scalar
            def _(scalar):
                scalar.wait_ge(w_sem, 16)
                scalar.activation(
                    wt[:], wc[:], mybir.ActivationFunctionType.Tanh
                ).then_inc(act_sem, 1)

            @block.vector
            def _(vector):
                # the DVE does not interlock same-engine RAW hazards:
                # every producer bumps p_sem and consumers wait on it
                vector.wait_ge(act_sem, 1)
                # a0 = 1 - relu(-w) = min(w + 1, 1)  (even segments, c=0)
                vector.tensor_scalar(
                    a0[:], wt[:], 1.0, 1.0, mybir.AluOpType.add, mybir.AluOpType.min
                ).then_inc(p_sem, 1)
                # a1 = 1 - relu(w) = min(1 - w, 1)   (odd segments, c=1)
                vector.tensor_scalar(
                    nw[:], wt[:], -1.0, 1.0, mybir.AluOpType.mult, mybir.AluOpType.add
                ).then_inc(p_sem, 1)
                vector.wait_ge(p_sem, 2)
                vector.tensor_scalar_min(a1[:], nw[:], 1.0).then_inc(p_sem, 1)
                # vals[p, t, s] = (a0 if s even else a1)[p, t]
                vector.wait_ge(p_sem, 3)
                for s in range(SEGS):
                    vector.tensor_copy(
                        vals[:, :, s], (a0 if s % 2 == 0 else a1)[:, :]
                    ).then_inc(p_sem, 1)
                vector.wait_ge(p_sem, 3 + SEGS)
                for t in range(N_TILES):
                    vector.wait_ge(in_sems[t], 16)
                    vector.tensor_copy(tiles[t][:, :, 0], vals[:, t, :]).then_inc(
                        dve_sem, 1
                    )

            @block.gpsimd
            def _(gpsimd):
                for t in range(N_TILES):
                    gpsimd.wait_ge(in_sems[t], 16)
                    gpsimd.wait_ge(dve_sem, t + 1)
                    gpsimd.dma_start(
                        out=out[t, :, :, :], in_=tiles[t][:, :, :]
                    ).then_inc(out_sem, 16)
                gpsimd.wait_ge(out_sem, 16 * N_TILES)

    return nc


def kernel(**inputs: np.ndarray) -> np.ndarray:
    from concourse.bass_utils import run_bass_kernel_spmd

    global _NC_CACHE, LAST_RESULT
    weight = np.asarray(inputs["weight"], dtype=np.float32)
    prev_a = np.ascontiguousarray(np.asarray(inputs["prev_a"], dtype=np.float32))
    step = int(np.asarray(inputs["step"]))

    wrow = weight[step]  # (G,) replicated weight row, sliced per core
    if _NC_CACHE is None:
        _NC_CACHE = _build()
    nc = _NC_CACHE

    shards = prev_a.reshape(M, N_TILES, 128, SEGS, L)
    in_maps = []
    for m in range(M):
        wc = np.ascontiguousarray(
            wrow[m * G_LOC : (m + 1) * G_LOC].reshape(N_TILES, 128).T
        )
        in_maps.append({"prev": np.ascontiguousarray(shards[m]), "wcol": wc})

    res = run_bass_kernel_spmd(nc, in_maps, core_ids=list(range(M)), trace=TRACE)
    if TRACE:
        LAST_RESULT = res
    outs = [np.asarray(res.results[m]["out"]).reshape(-1) for m in range(M)]
    return np.concatenate(outs)


# revision 12
# speedup vs baseline: 1.0045x; 1.0045x over previous
"""Trainium2 kernel for nn_AdaOpenController.

Reference semantics (G=4096 groups, P=4 pairs, 2 muscles, L=1024 dofs):
    w   = tanh(weight[step])                  # (G,)
    mu  = [relu(-w), relu(w)]                 # (G, 2) per-group segment heads
    out = 1 - concat([mu, (1-prev_a)[..., :-1]], -1)
i.e. for each of the G*P*2 length-L segments:
    out[seg, 0]  = 1 - mu[g, c]      (c = muscle index, segment head)
    out[seg, l]  = prev_a[seg, l-1]  (l >= 1; pure shift-by-one copy)

Memory-bound: 128 MiB in + 128 MiB out, no FLOPs to speak of. Sharded
data-parallel over the group axis G across 8 NeuronCores (16 MiB in/out
per core; the relevant weight-row slice is tiny and precomputed per
core on the host).

Per core (raw Bass; single 16 MiB SBUF tile):
  - one fully contiguous HWDGE read of the whole shard, landing at SBUF
    free-offset +1 — the shift-by-one falls out of the DMA layout;
    head slots (t*8192 + s*1024) hold junk afterwards
  - VectorE computes the 32 head values from tanh(weight row) and
    overwrites the junk slots with one strided copy
  - one fully contiguous SWDGE store of the tile
  Serial read-then-write keeps HBM in pure-read then pure-write phases
  (measured faster than interleaving), and both DMAs use 32 KiB per
  partition descriptors. Measured ~93 us/core vs the ~94 us HBM
  roofline (33.5 MB at 358 GB/s per core).

Raw Bass because walrus rejects queue DMAs with >1 embedded sync wait;
cross-engine ordering uses standalone wait_ge sequencer instructions,
and same-engine RAW chains go through p_sem (the DVE does not
interlock its own hazards).
"""

import sys

if "/opt/trn_rl_repo" not in sys.path:
    sys.path.insert(0, "/opt/trn_rl_repo")

from contextlib import ExitStack

import numpy as np

G = 4096
P = 4
L = 1024
M = 8
G_LOC = G // M           # 512
SEGS = P * 2             # 8
N_TILES = G_LOC // 128   # 4 chunks along the free dim
FREE = SEGS * L          # 8192 per chunk
LOC = G_LOC * SEGS * L

_NC_CACHE = None
TRACE = False
LAST_RESULT = None


def _build():
    import concourse.bass as bass
    import concourse.mybir as mybir

    dt = mybir.dt.float32
    nc = bass.Bass()
    prev = nc.declare_dram_parameter("prev", [N_TILES, 128, FREE], dt, isOutput=False)
    wcol = nc.declare_dram_parameter("wcol", [128, N_TILES], dt, isOutput=False)
    out = nc.declare_dram_parameter("out", [N_TILES, 128, FREE], dt, isOutput=True)

    with ExitStack() as ctx:
        ec = ctx.enter_context
        wc = ec(nc.sbuf_tensor("wc", [128, N_TILES], dt))
        wt = ec(nc.sbuf_tensor("wt", [128, N_TILES], dt))
        a0 = ec(nc.sbuf_tensor("a0", [128, N_TILES], dt))
        nw = ec(nc.sbuf_tensor("nw", [128, N_TILES], dt))
        a1 = ec(nc.sbuf_tensor("a1", [128, N_TILES], dt))
        vals = ec(nc.sbuf_tensor("vals", [128, N_TILES, SEGS], dt))
        tile = ec(nc.sbuf_tensor("tile", [128, N_TILES * FREE + 1], dt))
        w_sem = ec(nc.semaphore("w_sem"))
        act_sem = ec(nc.semaphore("act_sem"))
        in_sem = ec(nc.semaphore("in_sem"))
        p_sem = ec(nc.semaphore("p_sem"))
        dve_sem = ec(nc.semaphore("dve_sem"))
        out_sem = ec(nc.semaphore("out_sem"))

        with nc.Block() as block:

            @block.sync
            def _(sync):
                dst = tile[:, 1 : N_TILES * FREE + 1].rearrange(
                    "p (t f) -> p t f", t=N_TILES
                )
                sync.dma_start(
                    out=dst, in_=prev[:, :, :].rearrange("t p f -> p t f")
                ).then_inc(in_sem, 16)

            @block.scalar
            def _(scalar):
                scalar.wait_ge(w_sem, 16)
                scalar.activation(
                    wt[:], wc[:], mybir.ActivationFunctionType.Tanh
                ).then_inc(act_sem, 1)

            @block.vector
            def _(vector):
                vector.wait_ge(act_sem, 1)
                vector.tensor_scalar(
                    a0[:], wt[:], 1.0, 1.0, mybir.AluOpType.add, mybir.AluOpType.min
                ).then_inc(p_sem, 1)
                vector.tensor_scalar(
                    nw[:], wt[:], -1.0, 1.0, mybir.AluOpType.mult, mybir.AluOpType.add
                ).then_inc(p_sem, 1)
                vector.wait_ge(p_sem, 2)
                vector.tensor_scalar_min(a1[:], nw[:], 1.0).then_inc(p_sem, 1)
                vector.wait_ge(p_sem, 3)
                for s in range(SEGS):
                    vector.tensor_copy(
                        vals[:, :, s], (a0 if s % 2 == 0 else a1)[:, :]
                    ).then_inc(p_sem, 1)
                vector.wait_ge(p_sem, 3 + SEGS)
                vector.wait_ge(in_sem, 16)
                heads = tile[:, 0 : N_TILES * FREE].rearrange(
                    "p (t s l) -> p t s l", t=N_TILES, s=SEGS
                )
                vector.tensor_copy(heads[:, :, :, 0], vals[:, :, :]).then_inc(
                    dve_sem, 1
                )

            @block.gpsimd
            def _(gpsimd):
                gpsimd.dma_start(out=wc[:], in_=wcol[:, :]).then_inc(w_sem, 16)
                gpsimd.wait_ge(in_sem, 16)
                gpsimd.wait_ge(dve_sem, 1)
                osrc = tile[:, 0 : N_TILES * FREE].rearrange(
                    "p (t f) -> p t f", t=N_TILES
                )
                gpsimd.dma_start(
                    out=out[:, :, :].rearrange("t p f -> p t f"), in_=osrc
                ).then_inc(out_sem, 16)
                gpsimd.wait_ge(out_sem, 16)

    return nc


def kernel(**inputs: np.ndarray) -> np.ndarray:
    from concourse.bass_utils import run_bass_kernel_spmd

    global _NC_CACHE, LAST_RESULT
    weight = np.asarray(inputs["weight"], dtype=np.float32)
    prev_a = np.ascontiguousarray(np.asarray(inputs["prev_a"], dtype=np.float32))
    step = int(np.asarray(inputs["step"]))

    wrow = weight[step]
    if _NC_CACHE is None:
        _NC_CACHE = _build()
    nc = _NC_CACHE

    shards = prev_a.reshape(M, N_TILES, 128, FREE)
    in_maps = []
    for m in range(M):
        wc = np.ascontiguousarray(
            wrow[m * G_LOC : (m + 1) * G_LOC].reshape(N_TILES, 128).T
        )
        in_maps.append({"prev": np.ascontiguousarray(shards[m]), "wcol": wc})

    res = run_bass_kernel_spmd(nc, in_maps, core_ids=list(range(M)), trace=TRACE)
    if TRACE:
        LAST_RESULT = res
    outs = [np.asarray(res.results[m]["out"]).reshape(-1) for m in range(M)]
    return np.concatenate(outs)


# revision 15
# speedup vs baseline: 1.2532x; 1.2475x over previous
"""Trainium2 kernel for nn_AdaOpenController.

Reference semantics (G=4096 groups, P=4 pairs, 2 muscles, L=1024 dofs):
    w   = tanh(weight[step])                  # (G,)
    mu  = [relu(-w), relu(w)]                 # (G, 2) per-group segment heads
    out = 1 - concat([mu, (1-prev_a)[..., :-1]], -1)
i.e. for each of the G*P*2 length-L segments:
    out[seg, 0]  = 1 - mu[g, c]      (c = muscle index, segment head)
    out[seg, l]  = prev_a[seg, l-1]  (l >= 1; pure shift-by-one copy)

Memory-bound: 128 MiB in + 128 MiB out, no FLOPs to speak of. Sharded
data-parallel over the group axis G across 8 NeuronCores (16 MiB in/out
per core; the relevant weight-row slice is tiny and precomputed per
core on the host).

Per core (raw Bass; single 16 MiB SBUF tile):
  - two contiguous HWDGE read halves, landing at SBUF free-offset +1 —
    the shift-by-one falls out of the DMA layout; head slots
    (t*8192 + s*1024) hold junk afterwards
  - VectorE computes the 32 head values from tanh(weight row) and
    overwrites each half's junk slots as soon as that half lands
  - two contiguous SWDGE store halves, each issued as soon as its
    data + heads are in: the first store overlaps the second read
    half's drain, removing the mid-kernel read->write gap
  All DMAs use 32 KiB per partition descriptors. Measured 90.0 us/core
  best (~79-81 us of that is pure DMA at ~410-425 GB/s per NC); the
  rest is runtime startup, code fetch, and DMA receipt tails.

Raw Bass because walrus rejects queue DMAs with >1 embedded sync wait;
cross-engine ordering uses standalone wait_ge sequencer instructions,
and same-engine RAW chains go through p_sem (the DVE does not
interlock its own hazards). The Bass init-time all-engine barrier is
skipped (it only orders const-tile memsets; the one const consumer —
activation's float bias — is replaced by an explicitly synced zero
tile), and the Block exits with the cheap sem-only barrier (no gpsimd
dge_drain) since the out_sems already prove the stores landed.
"""

import sys

if "/opt/trn_rl_repo" not in sys.path:
    sys.path.insert(0, "/opt/trn_rl_repo")

from contextlib import ExitStack

import numpy as np

G = 4096
P = 4
L = 1024
M = 8
G_LOC = G // M           # 512
SEGS = P * 2             # 8
N_TILES = G_LOC // 128   # 4 chunks along the free dim
FREE = SEGS * L          # 8192 per chunk
LOC = G_LOC * SEGS * L

_NC_CACHE = None
TRACE = False
LAST_RESULT = None


def _build():
    import concourse.bass as bass
    import concourse.mybir as mybir

    dt = mybir.dt.float32
    # Bass.__init__ ends with 4 const-tile memsets + a full all-engine
    # barrier (~3.5us on HW). Nothing in this kernel reads the const APs
    # and all cross-engine ordering is via explicit semaphores, so skip it.
    _orig_barrier = bass.Bass.all_engine_barrier
    bass.Bass.all_engine_barrier = lambda self, *, sem_only=False: None
    try:
        nc = bass.Bass()
    finally:
        bass.Bass.all_engine_barrier = _orig_barrier
    prev = nc.declare_dram_parameter("prev", [N_TILES, 128, FREE], dt, isOutput=False)
    wcol = nc.declare_dram_parameter("wcol", [128, N_TILES], dt, isOutput=False)
    out = nc.declare_dram_parameter("out", [N_TILES, 128, FREE], dt, isOutput=True)

    with ExitStack() as ctx:
        ec = ctx.enter_context
        wc = ec(nc.sbuf_tensor("wc", [128, N_TILES], dt))
        zero = ec(nc.sbuf_tensor("zero", [128, 1], dt))
        wt = ec(nc.sbuf_tensor("wt", [128, N_TILES], dt))
        a0 = ec(nc.sbuf_tensor("a0", [128, N_TILES], dt))
        nw = ec(nc.sbuf_tensor("nw", [128, N_TILES], dt))
        a1 = ec(nc.sbuf_tensor("a1", [128, N_TILES], dt))
        vals = ec(nc.sbuf_tensor("vals", [128, N_TILES, SEGS], dt))
        tile = ec(nc.sbuf_tensor("tile", [128, N_TILES * FREE + 1], dt))
        w_sem = ec(nc.semaphore("w_sem"))
        z_sem = ec(nc.semaphore("z_sem"))
        act_sem = ec(nc.semaphore("act_sem"))
        in_sems = [ec(nc.semaphore(f"in_sem{h}")) for h in range(2)]
        p_sem = ec(nc.semaphore("p_sem"))
        dve_sem = ec(nc.semaphore("dve_sem"))
        out_sems = [ec(nc.semaphore(f"out_sem{h}")) for h in range(2)]

        # out_sem>=16 already guarantees the store fully landed; skip the
        # expensive gpsimd dge_drain and use the sem-only exit barrier
        with nc.Block(no_gpsimd_drain=True) as block:

            @block.sync
            def _(sync):
                HALF = N_TILES // 2
                for h in range(2):
                    lo, hi = h * HALF, (h + 1) * HALF
                    sync.dma_start(
                        out=tile[:, 1 + lo * FREE : 1 + hi * FREE].rearrange(
                            "p (t f) -> p t f", t=HALF
                        ),
                        in_=prev[lo:hi, :, :].rearrange("t p f -> p t f"),
                    ).then_inc(in_sems[h], 16)

            @block.scalar
            def _(scalar):
                scalar.wait_ge(z_sem, 1)
                scalar.wait_ge(w_sem, 16)
                scalar.activation(
                    wt[:], wc[:], mybir.ActivationFunctionType.Tanh, bias=zero[:, 0:1]
                ).then_inc(act_sem, 1)

            @block.vector
            def _(vector):
                vector.wait_ge(act_sem, 1)
                vector.tensor_scalar(
                    a0[:], wt[:], 1.0, 1.0, mybir.AluOpType.add, mybir.AluOpType.min
                ).then_inc(p_sem, 1)
                vector.tensor_scalar(
                    nw[:], wt[:], -1.0, 1.0, mybir.AluOpType.mult, mybir.AluOpType.add
                ).then_inc(p_sem, 1)
                vector.wait_ge(p_sem, 2)
                vector.tensor_scalar_min(a1[:], nw[:], 1.0).then_inc(p_sem, 1)
                vector.wait_ge(p_sem, 3)
                for s in range(SEGS):
                    vector.tensor_copy(
                        vals[:, :, s], (a0 if s % 2 == 0 else a1)[:, :]
                    ).then_inc(p_sem, 1)
                vector.wait_ge(p_sem, 3 + SEGS)
                heads = tile[:, 0 : N_TILES * FREE].rearrange(
                    "p (t s l) -> p t s l", t=N_TILES, s=SEGS
                )
                HALF = N_TILES // 2
                for h in range(2):
                    lo, hi = h * HALF, (h + 1) * HALF
                    vector.wait_ge(in_sems[h], 16)
                    vector.tensor_copy(
                        heads[:, lo:hi, :, 0], vals[:, lo:hi, :]
                    ).then_inc(dve_sem, 1)

            @block.gpsimd
            def _(gpsimd):
                # explicit zero bias for the activation (the default float
                # bias reads a const tile whose init barrier we removed)
                gpsimd.memset(zero[:], 0.0).then_inc(z_sem, 1)
                gpsimd.dma_start(out=wc[:], in_=wcol[:, :]).then_inc(w_sem, 16)
                # store each half as soon as its data + heads are in: the
                # first store overlaps the second read half's drain
                HALF = N_TILES // 2
                for h in range(2):
                    lo, hi = h * HALF, (h + 1) * HALF
                    gpsimd.wait_ge(in_sems[h], 16)
                    gpsimd.wait_ge(dve_sem, h + 1)
                    osrc = tile[:, lo * FREE : hi * FREE].rearrange(
                        "p (t f) -> p t f", t=HALF
                    )
                    gpsimd.dma_start(
                        out=out[lo:hi, :, :].rearrange("t p f -> p t f"), in_=osrc
                    ).then_inc(out_sems[h], 16)
                gpsimd.wait_ge(out_sems[0], 16)
                gpsimd.wait_ge(out_sems[1], 16)

    return nc


def kernel(**inputs: np.ndarray) -> np.ndarray:
    from concourse.bass_utils import run_bass_kernel_spmd

    global _NC_CACHE, LAST_RESULT
    weight = np.asarray(inputs["weight"], dtype=np.float32)
    prev_a = np.ascontiguousarray(np.asarray(inputs["prev_a"], dtype=np.float32))
    step = int(np.asarray(inputs["step"]))

    wrow = weight[step]
    if _NC_CACHE is None:
        _NC_CACHE = _build()
    nc = _NC_CACHE

    shards = prev_a.reshape(M, N_TILES, 128, FREE)
    in_maps = []
    for m in range(M):
        wc = np.ascontiguousarray(
            wrow[m * G_LOC : (m + 1) * G_LOC].reshape(N_TILES, 128).T
        )
        in_maps.append({"prev": np.ascontiguousarray(shards[m]), "wcol": wc})

    res = run_bass_kernel_spmd(nc, in_maps, core_ids=list(range(M)), trace=TRACE)
    if TRACE:
        LAST_RESULT = res
    outs = [np.asarray(res.results[m]["out"]).reshape(-1) for m in range(M)]
    return np.concatenate(outs)
